# revision 29
# baseline (speedup 1.0000x reference)
"""Trainium2 Bass kernel for DPAttention (attention block + residual + LayerNorm).

Sharding: 8 cores = DP2 (batch) x TP4 (head groups of 3 heads).
Core c: b = c//4, g = c%4 -> heads [3g, 3g+3).
Output rows of core g: [256g, 256g+256) u [1024+256g, 1024+256g+256)
(one quarter of each query half). Each query half gets its own 4-core
AllToAll of ctx^T (each peer only needs my 256 query columns).

Attention runs as 3 "superunits", each processing two (head, query-half)
streams concurrently: stream-lo on PE row group 0:64, stream-hi on rows
64:128 (K=64 matmuls at tile positions (0,0)/(64,0) run concurrently).
Per kt, the two 512-col sub-chunks land in two PSUM tiles; sub0 goes to
ScalarE exp, sub1 mostly to VectorE via a 1-instruction bf16-Schraudolph
exp (bits = scores*C1 + bias, converted to int16, bitcast to bf16).
e is kept in a 4-deep ring (consumed by ctx within the superunit).
ctx^T [d+1, q] = sum_k matmul(lhsT=[V|1], rhs=e) with a ones column for
the softmax denominator; a rank-1 (1e18 * u) matmul overwrites
invalid-query columns with the uniform-attention value u = mean_k V.

All weights are host-repacked so every DMA is one contiguous chunk per
partition (the DMA rings are descriptor-rate-bound at ~45 desc/us).
Head-2 K and Q projections are packed into one m=128 PE pass.
"""
import numpy as np
import ml_dtypes

import concourse.bass as bass
import concourse.mybir as mybir
import concourse.tile as tile
from concourse import bacc
from concourse.bass_utils import run_bass_kernel_spmd

F32 = mybir.dt.float32
BF16 = mybir.dt.bfloat16
U32 = mybir.dt.uint32
I16 = mybir.dt.int16
F8 = mybir.dt.float8e4
AF = mybir.ActivationFunctionType
ALU = mybir.AluOpType
AX = mybir.AxisListType

B, S, H, NH, HD = 2, 2048, 768, 12, 64
P = 128
KT = H // P            # 6 contraction tiles over hidden
ST = S // P            # 16 tiles over sequence
TP = 4                 # head groups (tensor-parallel within a batch)
HG = NH // TP          # 3 heads per core
HGD = HG * HD          # 192
HD1 = HD + 1           # V columns + denominator ones column
SQ = S // TP           # 512 output rows per core
QQ = SQ // 2           # 256 rows per query half
EPS = 1e-5
SCALE = 1.0 / np.sqrt(HD)
NCORES = 8
GROUPS = [[0, 1, 2, 3], [4, 5, 6, 7]]
BIGNEG = -1.0e9
BIGPOS = 1.0e18
# bf16 Schraudolph exp constants (DVE path): bits16 = s*SC_C1 + bias_k
SC_C1 = float(SCALE * np.log2(np.e) * 128.0)
SC_B0 = 16251.0          # 127*128 - 5.0 (tuned for truncating convert)
SC_BM = 1300.0           # masked keys -> e ~= 2^-117

LAG = 2

_cache = {}


def _route_dve(su, kt, sub):
    # sub-chunk 1 of each kt goes to the DVE exp, except every 8th kt
    return sub == 1 and (kt % 8) != 7


def build():
    nc = bacc.Bacc(num_devices=NCORES)

    xt_d = nc.dram_tensor("xt", [H, S], BF16, kind="ExternalInput")
    xres_d = nc.dram_tensor("xres", [P, (SQ // P) * H], F32, kind="ExternalInput")
    wk_d = nc.dram_tensor("wk", [P, KT * P], BF16, kind="ExternalInput")
    wq_d = nc.dram_tensor("wq", [P, KT * P], BF16, kind="ExternalInput")
    wkq2_d = nc.dram_tensor("wkq2", [P, KT * P], BF16, kind="ExternalInput")
    wv_d = nc.dram_tensor("wv", [P, KT * HGD], BF16, kind="ExternalInput")
    bvr_d = nc.dram_tensor("bvr", [P, HGD], F32, kind="ExternalInput")
    wo_d = nc.dram_tensor("wo", [P, KT * H], BF16, kind="ExternalInput")
    gq_d = nc.dram_tensor("gq", [1, S], BF16, kind="ExternalInput")
    # all small per-partition f32 tensors coalesced into one DMA
    # (each separate load costs 128 descriptors on a ~45 desc/us ring):
    # cols [0:4]=bqk [4:20]=mkb [20:36]=mkb2 [36:40]=xsum
    sm_d = nc.dram_tensor("sm", [P, 40], F32, kind="ExternalInput")
    qoff_d = nc.dram_tensor("qoff", [1, 1], U32, kind="ExternalInput")
    out_d = nc.dram_tensor("out", [SQ, H], F32, kind="ExternalOutput")

    from contextlib import ExitStack

    with tile.TileContext(nc) as tc:
        with (
            tc.tile_pool(name="wts", bufs=1) as wts,
            tc.tile_pool(name="qkv", bufs=1) as qkv,
            tc.tile_pool(name="dram", bufs=1, space="DRAM") as dram,
        ):
            qoff_sb = wts.tile([1, 1], U32)
            nc.gpsimd.dma_start(qoff_sb[:], qoff_d[:])

            # ---- critical-path loads first. All repacked: one contiguous
            # chunk per partition, so each load is ~128 descriptors.
            wk_sb = wts.tile([P, KT, P], BF16)
            nc.sync.dma_start(wk_sb[:], wk_d.rearrange("p (kt m) -> p kt m", m=P))
            wq_sb = wts.tile([P, KT, P], BF16)
            nc.scalar.dma_start(wq_sb[:], wq_d.rearrange("p (kt m) -> p kt m", m=P))

            # gpsimd ring: packed K2|Q2 weights + coalesced smalls
            wkq2_sb = wts.tile([P, KT, P], BF16)
            nc.gpsimd.dma_start(wkq2_sb[:], wkq2_d.rearrange("p (kt m) -> p kt m", m=P))
            sm_sb = wts.tile([P, 40], F32)
            nc.gpsimd.dma_start(sm_sb[:], sm_d[:])
            bqk_sb = sm_sb[:, 0:4]
            mkb_sb = sm_sb[:, 4:20]
            mkb2_sb = sm_sb[:, 20:36]
            xsum_sb = sm_sb[:, 36:40]
            gq_sb = wts.tile([1, S], BF16)
            nc.gpsimd.dma_start(gq_sb[:], gq_d[:])

            # scalar ring: V weights (needed ~10us in)
            wv_sb = wts.tile([P, KT, HGD], BF16)
            nc.scalar.dma_start(wv_sb[:], wv_d.rearrange("p (kt m) -> p kt m", m=HGD))
            bvr_sb = wts.tile([P, HG, HD], F32)
            nc.scalar.dma_start(bvr_sb[:], bvr_d.rearrange("p (h d) -> p h d", d=HD))

            # late-needed loads (out dense + residual epilogue)
            wo_sb = wts.tile([P, KT, H], BF16)
            nc.scalar.dma_start(wo_sb[:], wo_d.rearrange("p (kt n) -> p kt n", n=H))
            xres_sb = wts.tile([P, SQ // P, H], F32)
            nc.scalar.dma_start(xres_sb[:], xres_d.rearrange("p (t n) -> p t n", n=H))

            ones_sb = wts.tile([P, 1], BF16)
            nc.gpsimd.memset(ones_sb[:], 1.0)

            # ---- persistent intermediate tiles ----
            qt_sb = qkv.tile([P, S], BF16)    # Q^T h0 (rows 0:64), h1 (64:128)
            qt2_sb = qkv.tile([P, S], BF16)   # Q^T h2: rows 0:64 for half 0, 64:128 for half 1
            kt01_sb = qkv.tile([P, S], BF16)  # K^T h0 (rows 0:64), h1 (64:128)
            kt22_sb = qkv.tile([P, S], BF16)  # K^T h2 duplicated in both row halves
            v_sb = qkv.tile([P, ST, HG, HD1], BF16)   # V + ones col per head
            u_sb = qkv.tile([1, HG, HD1], BF16)       # mean_k V (+1 slot)
            ctxa_sb = qkv.tile([P, S], F8)   # ctx^T heads 0,1 (fp8: halves
            ctxb_sb = qkv.tile([HD, S], F8)  # the AllGather wire) + head 2
            ctxg_sb = [qkv.tile([P, KT, QQ], F8, name=f"ctxg{x}")
                       for x in range(2)]

            nc.gpsimd.memset(v_sb[:, :, :, HD:HD1], 1.0)

            # superunits: (lo stream rows 0:64, hi stream rows 64:128)
            SUS = [((0, 0), (1, 0)), ((2, 0), (2, 1)), ((0, 1), (1, 1))]
            KQ_OF_SU = [(kt01_sb, qt_sb), (kt22_sb, qt2_sb), (kt01_sb, qt_sb)]

            pools = ExitStack()
            epool = pools.enter_context(tc.tile_pool(name="epool", bufs=2))
            cps = pools.enter_context(tc.tile_pool(name="cps", bufs=2, space="PSUM"))
            npool = pools.enter_context(tc.tile_pool(name="npool", bufs=2))
            spsx = ExitStack()
            sps = spsx.enter_context(tc.tile_pool(name="sps", bufs=2, space="PSUM"))

            NST = SQ // P

            def emit_scores_kt(su, kt, er):
                (h_lo, qh_lo), (h_hi, qh_hi) = SUS[su]
                Kt, Qt = KQ_OF_SU[su]
                lhsT_lo = Kt[0:HD, kt * P:(kt + 1) * P]
                lhsT_hi = Kt[HD:P, kt * P:(kt + 1) * P]
                for sub in range(2):
                    ps = sps.tile([P, 1024], F32, tag="sc", name=f"sc{su}_{kt}_{sub}")
                    ql = qh_lo * 1024 + sub * 512
                    qh = qh_hi * 1024 + sub * 512
                    nc.tensor.matmul(ps[:, 0:512], lhsT_lo, Qt[0:HD, ql:ql + 512],
                                     start=True, stop=True)
                    nc.tensor.matmul(ps[:, 512:1024], lhsT_hi, Qt[HD:P, qh:qh + 512],
                                     start=True, stop=True)
                    if _route_dve(su, kt, sub):
                        nc.vector.tensor_scalar(
                            er[:, sub, :].bitcast(I16), ps[:],
                            SC_C1, mkb2_sb[:, kt:kt + 1],
                            op0=ALU.mult, op1=ALU.add)
                    else:
                        nc.scalar.activation(er[:, sub, :], ps[:], AF.Exp,
                                             bias=mkb_sb[:, kt:kt + 1],
                                             scale=float(SCALE))

            def emit_ctx_kt(su, kt, er, pc_lo, pc_hi):
                (h_lo, _), (h_hi, _) = SUS[su]
                for sub in range(2):
                    nc.tensor.matmul(
                        pc_lo[0:HD1, sub * 512:(sub + 1) * 512],
                        v_sb[:, kt, h_lo, :], er[:, sub, 0:512],
                        start=(kt == 0), stop=False)
                    nc.tensor.matmul(
                        pc_hi[0:HD1, sub * 512:(sub + 1) * 512],
                        v_sb[:, kt, h_hi, :], er[:, sub, 512:1024],
                        start=(kt == 0), stop=False)

            def emit_ctx_tails(su, pc_lo, pc_hi):
                # both streams' tails interleaved so the per-step latencies
                # (DVE single-partition ops + the DRAM broadcast bounce for
                # the denominator; stride-0 SBUF reads are illegal) overlap
                (h_lo, qh_lo), (h_hi, qh_hi) = SUS[su]
                streams = ((h_lo, qh_lo, pc_lo, "lo"), (h_hi, qh_hi, pc_hi, "hi"))
                for h, qh, pc, nm in streams:
                    q0 = qh * 1024
                    for sub in range(2):
                        nc.tensor.matmul(
                            pc[0:HD1, sub * 512:(sub + 1) * 512],
                            u_sb[0:1, h, :],
                            gq_sb[0:1, q0 + sub * 512:q0 + (sub + 1) * 512],
                            start=False, stop=True)
                dens, rbs = {}, {}
                for h, qh, pc, nm in streams:
                    den = npool.tile([1, 1024], F32, tag="den",
                                     name=f"den{su}{nm}")
                    nc.vector.tensor_copy(den[:], pc[HD:HD1, :])
                    dens[nm] = den
                for h, qh, pc, nm in streams:
                    nc.vector.reciprocal_approx_fast(dens[nm][:], dens[nm][:])
                for h, qh, pc, nm in streams:
                    rden = dram.tile([1, 1024], F32, tag="rden", bufs=2,
                                     name=f"rden{su}{nm}")
                    nc.gpsimd.dma_start(rden[:], dens[nm][:])
                    rb = npool.tile([HD, 1024], F32, tag="rb",
                                    name=f"rb{su}{nm}")
                    nc.gpsimd.dma_start(rb[:], rden[0:1, :].to_broadcast((HD, 1024)))
                    rbs[nm] = rb
                for h, qh, pc, nm in streams:
                    q0 = qh * 1024
                    dst = (ctxa_sb[HD * h:HD * (h + 1), q0:q0 + 1024] if h < 2
                           else ctxb_sb[:, q0:q0 + 1024])
                    nc.vector.tensor_tensor(dst, pc[0:HD, :], rbs[nm][:],
                                            op=ALU.mult)

            # per query half: AllGather of ctx^T across the TP group
            # (AllToAll would be 5x less wire but mesh needs >4 ranks)
            ag_in = [dram.tile([HGD, 1024], F8, name=f"agi{x}")
                     for x in range(2)]
            ag_out = [dram.tile([TP, HGD, 1024], F8, name=f"ago{x}")
                      for x in range(2)]

            def emit_ag_in01(qh, engs):
                # h0/h1 rows (0:128) of the AG payload for half qh
                q0 = qh * 1024
                engs[0].dma_start(ag_in[qh][0:HD, :],
                                  ctxa_sb[0:HD, q0:q0 + 1024])
                engs[-1].dma_start(ag_in[qh][HD:P, :],
                                   ctxa_sb[HD:P, q0:q0 + 1024])

            def emit_ag_in2(qh, eng):
                # h2 rows (128:192) of the AG payload for half qh
                q0 = qh * 1024
                eng.dma_start(ag_in[qh][P:HGD, :], ctxb_sb[:, q0:q0 + 1024])

            def emit_ag(qh):
                nc.gpsimd.collective_compute(
                    "AllGather", ALU.bypass, replica_groups=GROUPS,
                    ins=[ag_in[qh].opt()], outs=[ag_out[qh].opt()],
                )

            def emit_pull(qh, engs):
                # pull this core's 256 query columns of the gathered half,
                # split across DMA queues (the rings are desc-rate-bound)
                v = (ag_out[qh].rearrange("g d q -> (g d) q")
                     .rearrange("(kt p) q -> p kt q", p=P))
                for kt in range(KT):
                    eng = engs[kt % len(engs)]
                    eng.dma_start(ctxg_sb[qh][:, kt, :],
                                  v[:, kt, bass.ds(qi_v[QI[eng]], QQ)])

            # ======== projections (xt freed right after superunit 0) ========
            with tc.tile_pool(name="xt", bufs=1) as xtp:
                xt_sb = xtp.tile([P, KT, S], BF16)
                xt_r = xt_d.rearrange("(kt p) s -> p kt s", p=P)
                for kt in range(KT):
                    nc.sync.dma_start(xt_sb[:, kt, :], xt_r[:, kt, :])

                # K/Q projection chunk for heads 0,1 (m=128 passes); the
                # chunks are interleaved into superunit 0 below so scores
                # can start as soon as K qc0/qc1 + Q qc0/qc1 exist
                def emit_kq_chunk(w_sb, bc, d_sb, qc):
                    qs = slice(qc * 512, (qc + 1) * 512)
                    ps = sps.tile([P, 512], F32, tag="sc",
                                  name=f"pj{bc}_{qc}")
                    for kt in range(KT):
                        nc.tensor.matmul(
                            ps[:], w_sb[:, kt, :], xt_sb[:, kt, qs],
                            start=(kt == 0), stop=(kt == KT - 1),
                        )
                    nc.vector.tensor_scalar_add(
                        d_sb[:, qs], ps[:], bqk_sb[:, bc:bc + 1])

                emit_kq_chunk(wk_sb, 2, kt01_sb, 0)
                emit_kq_chunk(wq_sb, 0, qt_sb, 0)

                # snap this core's query offset (256*g) on every
                # DMA-capable engine in ONE critical section; its barrier
                # (every engine waits for the slowest to arrive) overlaps
                # the first projection chunk's matmuls
                qi_v = {}
                with tc.tile_critical():
                    for eng, nm in ((nc.gpsimd, "qig"), (nc.sync, "qis"),
                                    (nc.scalar, "qia")):
                        with eng.register(nm) as r:
                            eng.reg_load(r, qoff_sb[0:1, 0:1])
                            qi_v[nm] = eng.snap(r)
                QI = {nc.gpsimd: "qig", nc.sync: "qis", nc.scalar: "qia"}

                emit_kq_chunk(wk_sb, 2, kt01_sb, 1)
                emit_kq_chunk(wq_sb, 0, qt_sb, 1)

                def emit_kq2(qc):
                    # packed head-2 pass: rows 0:64 = Q2^T, 64:128 = K2^T.
                    # DVE partition shifts only go upward (src base <= dst
                    # base); K2's row 0:64 copy happens via DMA afterwards.
                    qs = slice(qc * 512, (qc + 1) * 512)
                    ps = sps.tile([P, 512], F32, tag="sc", name=f"kq2_{qc}")
                    for kt in range(KT):
                        nc.tensor.matmul(
                            ps[:], wkq2_sb[:, kt, :], xt_sb[:, kt, qs],
                            start=(kt == 0), stop=(kt == KT - 1),
                        )
                    # Q2^T: half 0 rows 0:64 (no shift), half 1 rows 64:128
                    dst = (qt2_sb[0:HD, qs] if qc < 2
                           else qt2_sb[HD:P, qs])
                    nc.vector.tensor_scalar_add(
                        dst, ps[0:HD], bqk_sb[0:HD, 1:2])
                    # K2^T into rows 64:128 (no shift)
                    nc.vector.tensor_scalar_add(
                        kt22_sb[HD:P, qs], ps[HD:P], bqk_sb[HD:P, 3:4])

                def emit_vproj(st):
                    ps = sps.tile([P, HGD], F32, tag="sc", name=f"vp{st}")
                    for kt in range(KT):
                        nc.tensor.matmul(
                            ps[:], xt_sb[:, kt, st * P:(st + 1) * P], wv_sb[:, kt, :],
                            start=(kt == 0), stop=(kt == KT - 1),
                        )
                    nc.vector.tensor_tensor(
                        v_sb[:, st, :, 0:HD], ps[:].rearrange("p (h d) -> p h d", d=HD),
                        bvr_sb[:], op=ALU.add,
                    )

                def emit_u():
                    ups = sps.tile([1, HGD], F32, tag="sc", name="ups")
                    for st in range(ST):
                        nc.tensor.matmul(
                            ups[:], ones_sb[:], v_sb[:, st, :, 0:HD],
                            start=(st == 0), stop=(st == ST - 1),
                        )
                    nc.vector.tensor_scalar_mul(
                        u_sb[0:1, :, 0:HD],
                        ups[:].rearrange("p (h d) -> p h d", d=HD), 1.0 / S)
                    nc.gpsimd.memset(u_sb[:, :, HD:HD1], 1.0)

                # ---- superunit 0 (V proj + head-2 K/Q proj interleaved) ----
                pc_lo = cps.tile([P, 1024], F32, tag="c", name="c0lo")
                pc_hi = cps.tile([P, 1024], F32, tag="c", name="c0hi")
                ers = {}
                for kt in range(ST):
                    # remaining projection chunks ride along with SU0
                    if kt == 0:
                        emit_kq_chunk(wk_sb, 2, kt01_sb, 2)
                    elif kt == 1:
                        emit_kq_chunk(wq_sb, 0, qt_sb, 2)
                    elif kt == 2:
                        emit_kq_chunk(wk_sb, 2, kt01_sb, 3)
                    elif kt == 3:
                        emit_kq_chunk(wq_sb, 0, qt_sb, 3)
                    elif kt < 8:
                        emit_kq2(kt - 4)
                    if kt == 8:
                        # duplicate K2^T into rows 0:64 for the lo stream
                        nc.gpsimd.dma_start(kt22_sb[0:HD, :], kt22_sb[HD:P, :])
                    emit_vproj(kt)
                    ers[kt] = epool.tile([P, 2, 1024], BF16, tag="e", bufs=4,
                                         name=f"e0_{kt}")
                    emit_scores_kt(0, kt, ers[kt])
                    if kt >= LAG:
                        emit_ctx_kt(0, kt - LAG, ers.pop(kt - LAG), pc_lo, pc_hi)
                for kt in range(ST - LAG, ST):
                    emit_ctx_kt(0, kt, ers.pop(kt), pc_lo, pc_hi)
                emit_u()
                emit_ctx_tails(0, pc_lo, pc_hi)
                # h0/h1 of half 0 done -> stage their AG(0) rows (hidden
                # under SU1; sync ring is idle now)
                emit_ag_in01(0, [nc.sync, nc.sync])

            # ---- superunits 1, 2 ----
            for su in (1, 2):
                pc_lo = cps.tile([P, 1024], F32, tag="c", name=f"c{su}lo")
                pc_hi = cps.tile([P, 1024], F32, tag="c", name=f"c{su}hi")
                ers = {}
                for kt in range(ST):
                    ers[kt] = epool.tile([P, 2, 1024], BF16, tag="e", bufs=4,
                                         name=f"e{su}_{kt}")
                    emit_scores_kt(su, kt, ers[kt])
                    if kt >= LAG:
                        emit_ctx_kt(su, kt - LAG, ers.pop(kt - LAG), pc_lo, pc_hi)
                for kt in range(ST - LAG, ST):
                    emit_ctx_kt(su, kt, ers.pop(kt), pc_lo, pc_hi)
                emit_ctx_tails(su, pc_lo, pc_hi)
                if su == 1:
                    # h2 of both halves done: finish half-0 payload and
                    # fire AG(0); its pull is deliberately NOT queued yet
                    # (it would gate later same-queue DMAs on AG(0)'s
                    # completion and stall the SU2 tails).
                    emit_ag_in2(0, nc.gpsimd)
                    emit_ag(0)
                    emit_ag_in2(1, nc.scalar)
            # SU2 tails done: finish half-1 payload and fire AG(1)
            emit_ag_in01(1, [nc.sync, nc.scalar])
            emit_ag(1)
            emit_pull(0, [nc.sync, nc.scalar, nc.gpsimd])

            spsx.close()   # free scores PSUM banks for the out-dense

            # ======== out dense + residual + LayerNorm (per half) ========
            ops = pools.enter_context(tc.tile_pool(name="ops", bufs=2, space="PSUM"))
            lnp = pools.enter_context(tc.tile_pool(name="lnp", bufs=1))
            h_all = lnp.tile([P, NST, H], F32)
            mu_all = lnp.tile([P, NST], F32)
            var_all = lnp.tile([P, NST], F32)
            negmu = lnp.tile([P, NST], F32)
            rstd = lnp.tile([P, NST], F32)

            def emit_dense(st4):
                ps = ops.tile([P, H], F32, tag="od", name=f"od{st4}")
                src = ctxg_sb[st4 // 2]
                c0 = (st4 % 2) * P
                for kt in range(KT):
                    lhsT = src[:, kt, c0:c0 + P]
                    nc.tensor.matmul(ps[:, 0:512], lhsT, wo_sb[:, kt, 0:512],
                                     start=(kt == 0), stop=(kt == KT - 1))
                    nc.tensor.matmul(ps[:, 512:H], lhsT, wo_sb[:, kt, 512:H],
                                     start=(kt == 0), stop=(kt == KT - 1))
                # h = out_dense + (x + bo); mean via ACT copy-accumulate
                psc = lnp.tile([P, H], F32, tag="psc", bufs=2, name=f"psc{st4}")
                nc.scalar.activation(psc[:], ps[:], AF.Identity,
                                     accum_out=mu_all[:, st4:st4 + 1])
                nc.vector.tensor_tensor(h_all[:, st4, :], psc[:],
                                        xres_sb[:, st4, :], op=ALU.add)

            def emit_ln(sts):
                s0, s1 = sts[0], sts[0] + len(sts)
                sl = slice(s0, s1)
                sq_tmp = lnp.tile([P, H], F32, tag="sq", bufs=2, name=f"sq{s0}")
                nc.vector.tensor_tensor(mu_all[:, sl], mu_all[:, sl],
                                        xsum_sb[:, sl], op=ALU.add)
                nc.vector.tensor_scalar_mul(mu_all[:, sl], mu_all[:, sl], 1.0 / H)
                nc.vector.tensor_scalar_mul(negmu[:, sl], mu_all[:, sl], -1.0)
                for st4 in sts:
                    nc.scalar.activation(sq_tmp[:], h_all[:, st4, :], AF.Square,
                                         bias=negmu[:, st4:st4 + 1],
                                         accum_out=var_all[:, st4:st4 + 1])
                nc.vector.tensor_scalar_mul(var_all[:, sl], var_all[:, sl], 1.0 / H)
                nc.vector.tensor_scalar_add(var_all[:, sl], var_all[:, sl], EPS)
                # rstd = 1/sqrt(var) with one Newton step
                std0 = lnp.tile([P, 2], F32, tag="sd", bufs=2, name=f"sd{s0}")
                nc.scalar.activation(std0[:], var_all[:, sl], AF.Sqrt)
                y0 = lnp.tile([P, 2], F32, tag="y0", bufs=2, name=f"y0{s0}")
                nc.vector.reciprocal(y0[:], std0[:])
                t0 = lnp.tile([P, 2], F32, tag="t0", bufs=2, name=f"t0{s0}")
                nc.vector.tensor_tensor(t0[:], y0[:], y0[:], op=ALU.mult)
                nc.vector.tensor_tensor(t0[:], t0[:], var_all[:, sl], op=ALU.mult)
                nc.vector.tensor_scalar_mul(t0[:], t0[:], -0.5)
                nc.vector.tensor_scalar_add(t0[:], t0[:], 1.5)
                nc.vector.tensor_tensor(rstd[:, sl], y0[:], t0[:], op=ALU.mult)
                # out = h*rstd + (-mu*rstd) in one ACT pass (gamma=1 and
                # beta=0 for this model, asserted host-side)
                nmr = lnp.tile([P, 2], F32, tag="nmr", bufs=2, name=f"nmr{s0}")
                nc.vector.tensor_tensor(nmr[:], negmu[:, sl], rstd[:, sl],
                                        op=ALU.mult)
                for st4 in sts:
                    o_sb = lnp.tile([P, H], F32, tag="o", bufs=2, name=f"o{st4}")
                    nc.scalar.activation(o_sb[:], h_all[:, st4, :], AF.Identity,
                                         bias=nmr[:, st4 - s0:st4 - s0 + 1],
                                         scale=rstd[:, st4:st4 + 1])
                    nc.sync.dma_start(out_d[st4 * P:(st4 + 1) * P, :], o_sb[:])

            # half 0 (already pulled mid-SU2) overlaps A2A(1) flight
            emit_dense(0)
            emit_dense(1)
            emit_ln([0, 1])
            emit_pull(1, [nc.sync, nc.scalar, nc.gpsimd])
            emit_dense(2)
            emit_dense(3)
            emit_ln([2, 3])

            pools.close()

    nc.compile()
    return nc


def _rows(g):
    return np.r_[QQ * g:QQ * (g + 1), 1024 + QQ * g:1024 + QQ * (g + 1)]


def _repack_kt(w):
    # [KT*P, M] -> [P, KT*M]: row p holds kt-major chunks, so the SBUF
    # tile [P, KT, M] loads as one contiguous chunk per partition
    m = w.shape[1]
    return np.ascontiguousarray(
        w.reshape(KT, P, m).transpose(1, 0, 2).reshape(P, KT * m))


def _prep_inputs(inputs):
    hs = np.asarray(inputs["hidden_states"], dtype=np.float32)
    am = np.asarray(inputs["attention_mask"], dtype=np.float32)
    Wq = np.asarray(inputs["Wq"], dtype=np.float32)
    Wk = np.asarray(inputs["Wk"], dtype=np.float32)
    Wv = np.asarray(inputs["Wv"], dtype=np.float32)
    Wo = np.asarray(inputs["Wo"], dtype=np.float32)
    bq = np.asarray(inputs["bq"], dtype=np.float32)
    bk = np.asarray(inputs["bk"], dtype=np.float32)
    bv = np.asarray(inputs["bv"], dtype=np.float32)
    bo = np.asarray(inputs["bo"], dtype=np.float32)
    lng = np.asarray(inputs["ln_gamma"], dtype=np.float32)
    lnb = np.asarray(inputs["ln_beta"], dtype=np.float32)

    wo_bf = _repack_kt(Wo.astype(ml_dtypes.bfloat16))
    # the fused LN output path folds gamma/beta away (they are constant
    # identity in this model)
    assert np.all(lng == 1.0) and np.all(lnb == 0.0)

    in_maps = []
    for c in range(NCORES):
        b, g = c // TP, c % TP
        c0 = HGD * g
        valid = am[b] >= 0
        mk = np.where(valid, 0.0, BIGNEG).astype(np.float32)
        mk2 = np.where(valid, SC_B0, SC_BM).astype(np.float32)
        gqv = np.where(valid, 0.0, BIGPOS).astype(ml_dtypes.bfloat16)[None, :]
        rows = _rows(g)
        xres = hs[b, rows] + bo
        # bias columns: bq01 | bq2 (rows 0:64) | bk01 | bk2 (rows 64:128)
        bqk = np.zeros((P, 4), dtype=np.float32)
        bqk[:, 0] = bq[c0:c0 + P]
        bqk[0:HD, 1] = bq[c0 + P:c0 + HGD]
        bqk[:, 2] = bk[c0:c0 + P]
        bqk[HD:P, 3] = bk[c0 + P:c0 + HGD]
        wkq2 = np.concatenate(
            [Wq[:, c0 + P:c0 + HGD], Wk[:, c0 + P:c0 + HGD]], axis=1)
        sm = np.empty((P, 40), dtype=np.float32)
        sm[:, 0:4] = bqk
        sm[:, 4:20] = mk.reshape(ST, P).T
        sm[:, 20:36] = mk2.reshape(ST, P).T
        sm[:, 36:40] = xres.sum(axis=1).astype(np.float32).reshape(SQ // P, P).T
        in_maps.append({
            "xt": np.ascontiguousarray(hs[b].T).astype(ml_dtypes.bfloat16),
            "xres": np.ascontiguousarray(
                xres.reshape(SQ // P, P, H).transpose(1, 0, 2)
                .reshape(P, (SQ // P) * H)),
            "wq": _repack_kt(Wq[:, c0:c0 + P].astype(ml_dtypes.bfloat16)),
            "wk": _repack_kt(Wk[:, c0:c0 + P].astype(ml_dtypes.bfloat16)),
            "wkq2": _repack_kt(wkq2.astype(ml_dtypes.bfloat16)),
            "wv": _repack_kt(Wv[:, c0:c0 + HGD].astype(ml_dtypes.bfloat16)),
            "bvr": np.ascontiguousarray(np.broadcast_to(bv[c0:c0 + HGD], (P, HGD))),
            "wo": wo_bf,
            "gq": np.ascontiguousarray(gqv),
            "sm": sm,
            "qoff": np.array([[QQ * g]], dtype=np.uint32),
        })
    return in_maps


def _run(inputs, trace=False, trace_cores=None):
    if "nc" not in _cache:
        _cache["nc"] = build()
    nc = _cache["nc"]
    in_maps = _prep_inputs(inputs)
    res = run_bass_kernel_spmd(
        nc, in_maps, list(range(NCORES)), trace=trace,
        trace_cores=trace_cores,
    )
    out = np.empty((B, S, H), dtype=np.float32)
    for c in range(NCORES):
        b, g = c // TP, c % TP
        out[b, _rows(g)] = res.results[c]["out"]
    return out, res


def kernel(**inputs) -> np.ndarray:
    out, _ = _run(inputs)
    return out


# revision 37
# speedup vs baseline: 1.1078x; 1.1078x over previous
"""Trainium2 Bass kernel for DPAttention (attention block + residual + LayerNorm).

Sharding: 8 cores = DP2 (batch) x TP4 (head groups of 3 heads).
Core c: b = c//4, g = c%4 -> heads [3g, 3g+3).
Output rows of core g: [256g, 256g+256) u [1024+256g, 1024+256g+256)
(one quarter of each query half). Each query half gets its own 4-core
AllToAll of ctx^T (each peer only needs my 256 query columns).

Attention runs as 3 "superunits", each processing two (head, query-half)
streams concurrently: stream-lo on PE row group 0:64, stream-hi on rows
64:128 (K=64 matmuls at tile positions (0,0)/(64,0) run concurrently).
Per kt, the two 512-col sub-chunks land in two PSUM tiles; sub0 goes to
ScalarE exp, sub1 mostly to VectorE via a 1-instruction bf16-Schraudolph
exp (bits = scores*C1 + bias, converted to int16, bitcast to bf16).
e is kept in a 4-deep ring (consumed by ctx within the superunit).
ctx^T [d+1, q] = sum_k matmul(lhsT=[V|1], rhs=e) with a ones column for
the softmax denominator; a rank-1 (1e18 * u) matmul overwrites
invalid-query columns with the uniform-attention value u = mean_k V.

All weights are host-repacked so every DMA is one contiguous chunk per
partition (the DMA rings are descriptor-rate-bound at ~45 desc/us).
Head-2 K and Q projections are packed into one m=128 PE pass.
"""
import numpy as np
import ml_dtypes

import concourse.bass as bass
import concourse.mybir as mybir
import concourse.tile as tile
from concourse import bacc
from concourse.bass_utils import run_bass_kernel_spmd

F32 = mybir.dt.float32
BF16 = mybir.dt.bfloat16
U32 = mybir.dt.uint32
I16 = mybir.dt.int16
F8 = mybir.dt.float8e4
AF = mybir.ActivationFunctionType
ALU = mybir.AluOpType
AX = mybir.AxisListType

B, S, H, NH, HD = 2, 2048, 768, 12, 64
P = 128
KT = H // P            # 6 contraction tiles over hidden
ST = S // P            # 16 tiles over sequence
TP = 4                 # head groups (tensor-parallel within a batch)
HG = NH // TP          # 3 heads per core
HGD = HG * HD          # 192
HD1 = HD + 1           # V columns + denominator ones column
SQ = S // TP           # 512 output rows per core
QQ = SQ // 2           # 256 rows per query half
EPS = 1e-5
SCALE = 1.0 / np.sqrt(HD)
NCORES = 8
GROUPS = [[0, 1, 2, 3], [4, 5, 6, 7]]
BIGNEG = -1.0e9
BIGPOS = 1.0e18
# bf16 Schraudolph exp constants (DVE path): bits16 = s*SC_C1 + bias_k
SC_C1 = float(SCALE * np.log2(np.e) * 128.0)
SC_B0 = 16251.0          # 127*128 - 5.0 (tuned for truncating convert)
SC_BM = 1300.0           # masked keys -> e ~= 2^-117

LAG = 2

_cache = {}


def _route_dve(su, kt, sub):
    # sub-chunk 1 of each kt goes to the DVE exp, except every 8th kt
    return sub == 1 and (kt % 8) != 7


def build():
    nc = bacc.Bacc(num_devices=NCORES)

    xt_d = nc.dram_tensor("xt", [H, S], BF16, kind="ExternalInput")
    xres_d = nc.dram_tensor("xres", [P, (SQ // P) * H], F32, kind="ExternalInput")
    wk_d = nc.dram_tensor("wk", [P, KT * P], BF16, kind="ExternalInput")
    wq_d = nc.dram_tensor("wq", [P, KT * P], BF16, kind="ExternalInput")
    wkq2_d = nc.dram_tensor("wkq2", [P, KT * P], BF16, kind="ExternalInput")
    wv_d = nc.dram_tensor("wv", [P, KT * HGD], BF16, kind="ExternalInput")
    bvr_d = nc.dram_tensor("bvr", [P, HGD], F32, kind="ExternalInput")
    wo_d = nc.dram_tensor("wo", [P, KT * H], BF16, kind="ExternalInput")
    gq_d = nc.dram_tensor("gq", [1, S], BF16, kind="ExternalInput")
    # all small per-partition f32 tensors coalesced into one DMA
    # (each separate load costs 128 descriptors on a ~45 desc/us ring):
    # cols [0:4]=bqk [4:20]=mkb [20:36]=mkb2 [36:40]=xsum
    sm_d = nc.dram_tensor("sm", [P, 40], F32, kind="ExternalInput")
    qoff_d = nc.dram_tensor("qoff", [1, 1], U32, kind="ExternalInput")
    out_d = nc.dram_tensor("out", [SQ, H], F32, kind="ExternalOutput")

    from contextlib import ExitStack

    with tile.TileContext(nc) as tc:
        with (
            tc.tile_pool(name="wts", bufs=1) as wts,
            tc.tile_pool(name="qkv", bufs=1) as qkv,
            tc.tile_pool(name="dram", bufs=1, space="DRAM") as dram,
        ):
            qoff_sb = wts.tile([1, 1], U32)
            nc.gpsimd.dma_start(qoff_sb[:], qoff_d[:])

            # ---- critical-path loads first. All repacked: one contiguous
            # chunk per partition, so each load is ~128 descriptors.
            wk_sb = wts.tile([P, KT, P], BF16)
            nc.sync.dma_start(wk_sb[:], wk_d.rearrange("p (kt m) -> p kt m", m=P))
            wq_sb = wts.tile([P, KT, P], BF16)
            nc.scalar.dma_start(wq_sb[:], wq_d.rearrange("p (kt m) -> p kt m", m=P))

            # gpsimd ring: packed K2|Q2 weights + coalesced smalls
            wkq2_sb = wts.tile([P, KT, P], BF16)
            nc.gpsimd.dma_start(wkq2_sb[:], wkq2_d.rearrange("p (kt m) -> p kt m", m=P))
            sm_sb = wts.tile([P, 40], F32)
            nc.gpsimd.dma_start(sm_sb[:], sm_d[:])
            bqk_sb = sm_sb[:, 0:4]
            mkb_sb = sm_sb[:, 4:20]
            mkb2_sb = sm_sb[:, 20:36]
            xsum_sb = sm_sb[:, 36:40]
            gq_sb = wts.tile([1, S], BF16)
            nc.gpsimd.dma_start(gq_sb[:], gq_d[:])

            # scalar ring: V weights (needed ~10us in)
            wv_sb = wts.tile([P, KT, HGD], BF16)
            nc.scalar.dma_start(wv_sb[:], wv_d.rearrange("p (kt m) -> p kt m", m=HGD))
            bvr_sb = wts.tile([P, HG, HD], F32)
            nc.scalar.dma_start(bvr_sb[:], bvr_d.rearrange("p (h d) -> p h d", d=HD))

            # late-needed loads (out dense + residual epilogue)
            wo_sb = wts.tile([P, KT, H], BF16)
            nc.scalar.dma_start(wo_sb[:], wo_d.rearrange("p (kt n) -> p kt n", n=H))
            xres_sb = wts.tile([P, SQ // P, H], F32)
            nc.scalar.dma_start(xres_sb[:], xres_d.rearrange("p (t n) -> p t n", n=H))

            ones_sb = wts.tile([P, 1], BF16)
            nc.gpsimd.memset(ones_sb[:], 1.0)

            # ---- persistent intermediate tiles ----
            qt_sb = qkv.tile([P, S], BF16)    # Q^T h0 (rows 0:64), h1 (64:128)
            qt2_sb = qkv.tile([P, S], BF16)   # Q^T h2: rows 0:64 for half 0, 64:128 for half 1
            kt01_sb = qkv.tile([P, S], BF16)  # K^T h0 (rows 0:64), h1 (64:128)
            kt22_sb = qkv.tile([P, S], BF16)  # K^T h2 duplicated in both row halves
            v_sb = qkv.tile([P, ST, HG, HD1], BF16)   # V + ones col per head
            u_sb = qkv.tile([1, HG, HD1], BF16)       # mean_k V (+1 slot)
            ctxa_sb = qkv.tile([P, S], F8)   # ctx^T heads 0,1 (fp8: halves
            ctxb_sb = qkv.tile([HD, S], F8)  # the AllGather wire) + head 2
            ctxg_sb = [qkv.tile([P, KT, QQ], F8, name=f"ctxg{x}")
                       for x in range(2)]

            nc.gpsimd.memset(v_sb[:, :, :, HD:HD1], 1.0)

            # superunits: (lo stream rows 0:64, hi stream rows 64:128)
            SUS = [((0, 0), (1, 0)), ((2, 0), (2, 1)), ((0, 1), (1, 1))]
            KQ_OF_SU = [(kt01_sb, qt_sb), (kt22_sb, qt2_sb), (kt01_sb, qt_sb)]

            pools = ExitStack()
            epool = pools.enter_context(tc.tile_pool(name="epool", bufs=2))
            cps = pools.enter_context(tc.tile_pool(name="cps", bufs=2, space="PSUM"))
            npool = pools.enter_context(tc.tile_pool(name="npool", bufs=2))
            spsx = ExitStack()
            sps = spsx.enter_context(tc.tile_pool(name="sps", bufs=2, space="PSUM"))

            NST = SQ // P

            def emit_scores_kt(su, kt, er):
                (h_lo, qh_lo), (h_hi, qh_hi) = SUS[su]
                Kt, Qt = KQ_OF_SU[su]
                lhsT_lo = Kt[0:HD, kt * P:(kt + 1) * P]
                lhsT_hi = Kt[HD:P, kt * P:(kt + 1) * P]
                for sub in range(2):
                    ps = sps.tile([P, 1024], F32, tag="sc", name=f"sc{su}_{kt}_{sub}")
                    ql = qh_lo * 1024 + sub * 512
                    qh = qh_hi * 1024 + sub * 512
                    nc.tensor.matmul(ps[:, 0:512], lhsT_lo, Qt[0:HD, ql:ql + 512],
                                     start=True, stop=True)
                    nc.tensor.matmul(ps[:, 512:1024], lhsT_hi, Qt[HD:P, qh:qh + 512],
                                     start=True, stop=True)
                    if _route_dve(su, kt, sub):
                        nc.vector.tensor_scalar(
                            er[:, sub, :].bitcast(I16), ps[:],
                            SC_C1, mkb2_sb[:, kt:kt + 1],
                            op0=ALU.mult, op1=ALU.add)
                    else:
                        nc.scalar.activation(er[:, sub, :], ps[:], AF.Exp,
                                             bias=mkb_sb[:, kt:kt + 1],
                                             scale=float(SCALE))

            def emit_ctx_kt(su, kt, er, pc_lo, pc_hi):
                (h_lo, _), (h_hi, _) = SUS[su]
                for sub in range(2):
                    nc.tensor.matmul(
                        pc_lo[0:HD1, sub * 512:(sub + 1) * 512],
                        v_sb[:, kt, h_lo, :], er[:, sub, 0:512],
                        start=(kt == 0), stop=False)
                    nc.tensor.matmul(
                        pc_hi[0:HD1, sub * 512:(sub + 1) * 512],
                        v_sb[:, kt, h_hi, :], er[:, sub, 512:1024],
                        start=(kt == 0), stop=False)

            def emit_ctx_tails(su, pc_lo, pc_hi, teng=None):
                teng = teng or nc.gpsimd
                # both streams' tails interleaved so the per-step latencies
                # (DVE single-partition ops + the DRAM broadcast bounce for
                # the denominator; stride-0 SBUF reads are illegal) overlap
                (h_lo, qh_lo), (h_hi, qh_hi) = SUS[su]
                streams = ((h_lo, qh_lo, pc_lo, "lo"), (h_hi, qh_hi, pc_hi, "hi"))
                for h, qh, pc, nm in streams:
                    q0 = qh * 1024
                    for sub in range(2):
                        nc.tensor.matmul(
                            pc[0:HD1, sub * 512:(sub + 1) * 512],
                            u_sb[0:1, h, :],
                            gq_sb[0:1, q0 + sub * 512:q0 + (sub + 1) * 512],
                            start=False, stop=True)
                dens, rbs = {}, {}
                for h, qh, pc, nm in streams:
                    den = npool.tile([1, 1024], F32, tag="den",
                                     name=f"den{su}{nm}")
                    nc.vector.tensor_copy(den[:], pc[HD:HD1, :])
                    dens[nm] = den
                for h, qh, pc, nm in streams:
                    nc.vector.reciprocal_approx_fast(dens[nm][:], dens[nm][:])
                for h, qh, pc, nm in streams:
                    rden = dram.tile([1, 1024], F32, tag="rden", bufs=2,
                                     name=f"rden{su}{nm}")
                    teng.dma_start(rden[:], dens[nm][:])
                    rb = npool.tile([HD, 1024], F32, tag="rb",
                                    name=f"rb{su}{nm}")
                    teng.dma_start(rb[:], rden[0:1, :].to_broadcast((HD, 1024)))
                    rbs[nm] = rb
                for h, qh, pc, nm in streams:
                    q0 = qh * 1024
                    dst = (ctxa_sb[HD * h:HD * (h + 1), q0:q0 + 1024] if h < 2
                           else ctxb_sb[:, q0:q0 + 1024])
                    nc.vector.tensor_tensor(dst, pc[0:HD, :], rbs[nm][:],
                                            op=ALU.mult)

            # per query half: AllGather of ctx^T across the TP group
            # (AllToAll would be 5x less wire but mesh needs >4 ranks)
            ag_in = [dram.tile([HGD, 1024], F8, name=f"agi{x}")
                     for x in range(2)]
            ag_out = [dram.tile([TP, HGD, 1024], F8, name=f"ago{x}")
                      for x in range(2)]

            def emit_ag_in01(qh, engs):
                # h0/h1 rows (0:128) of the AG payload for half qh
                q0 = qh * 1024
                engs[0].dma_start(ag_in[qh][0:HD, :],
                                  ctxa_sb[0:HD, q0:q0 + 1024])
                engs[-1].dma_start(ag_in[qh][HD:P, :],
                                   ctxa_sb[HD:P, q0:q0 + 1024])

            def emit_ag_in2(qh, eng):
                # h2 rows (128:192) of the AG payload for half qh
                q0 = qh * 1024
                eng.dma_start(ag_in[qh][P:HGD, :], ctxb_sb[:, q0:q0 + 1024])

            def emit_ag(qh):
                nc.gpsimd.collective_compute(
                    "AllGather", ALU.bypass, replica_groups=GROUPS,
                    ins=[ag_in[qh].opt()], outs=[ag_out[qh].opt()],
                )

            def emit_pull(qh, engs):
                # pull this core's 256 query columns of the gathered half,
                # split across DMA queues (the rings are desc-rate-bound)
                v = (ag_out[qh].rearrange("g d q -> (g d) q")
                     .rearrange("(kt p) q -> p kt q", p=P))
                for kt in range(KT):
                    eng = engs[kt % len(engs)]
                    eng.dma_start(ctxg_sb[qh][:, kt, :],
                                  v[:, kt, bass.ds(qi_v[QI[eng]], QQ)])

            # ======== projections (xt freed right after superunit 0) ========
            with tc.tile_pool(name="xt", bufs=1) as xtp:
                xt_sb = xtp.tile([P, KT, S], BF16)
                xt_r = xt_d.rearrange("(kt p) s -> p kt s", p=P)
                for kt in range(KT):
                    nc.sync.dma_start(xt_sb[:, kt, :], xt_r[:, kt, :])

                # K/Q projection chunk for heads 0,1 (m=128 passes); the
                # chunks are interleaved into superunit 0 below so scores
                # can start as soon as K qc0/qc1 + Q qc0/qc1 exist
                def emit_kq_chunk(w_sb, bc, d_sb, qc):
                    qs = slice(qc * 512, (qc + 1) * 512)
                    ps = sps.tile([P, 512], F32, tag="sc",
                                  name=f"pj{bc}_{qc}")
                    for kt in range(KT):
                        nc.tensor.matmul(
                            ps[:], w_sb[:, kt, :], xt_sb[:, kt, qs],
                            start=(kt == 0), stop=(kt == KT - 1),
                        )
                    nc.vector.tensor_scalar_add(
                        d_sb[:, qs], ps[:], bqk_sb[:, bc:bc + 1])

                emit_kq_chunk(wk_sb, 2, kt01_sb, 0)
                emit_kq_chunk(wq_sb, 0, qt_sb, 0)

                # snap this core's query offset (256*g) on every
                # DMA-capable engine in ONE critical section; its barrier
                # (every engine waits for the slowest to arrive) overlaps
                # the first projection chunk's matmuls
                qi_v = {}
                with tc.tile_critical():
                    for eng, nm in ((nc.gpsimd, "qig"), (nc.sync, "qis"),
                                    (nc.scalar, "qia")):
                        with eng.register(nm) as r:
                            eng.reg_load(r, qoff_sb[0:1, 0:1])
                            qi_v[nm] = eng.snap(r)
                QI = {nc.gpsimd: "qig", nc.sync: "qis", nc.scalar: "qia"}

                emit_kq_chunk(wk_sb, 2, kt01_sb, 1)
                emit_kq_chunk(wq_sb, 0, qt_sb, 1)

                def emit_kq2(qc):
                    # packed head-2 pass: rows 0:64 = Q2^T, 64:128 = K2^T.
                    # DVE partition shifts only go upward (src base <= dst
                    # base); K2's row 0:64 copy happens via DMA afterwards.
                    qs = slice(qc * 512, (qc + 1) * 512)
                    ps = sps.tile([P, 512], F32, tag="sc", name=f"kq2_{qc}")
                    for kt in range(KT):
                        nc.tensor.matmul(
                            ps[:], wkq2_sb[:, kt, :], xt_sb[:, kt, qs],
                            start=(kt == 0), stop=(kt == KT - 1),
                        )
                    # Q2^T: half 0 rows 0:64 (no shift), half 1 rows 64:128
                    dst = (qt2_sb[0:HD, qs] if qc < 2
                           else qt2_sb[HD:P, qs])
                    nc.vector.tensor_scalar_add(
                        dst, ps[0:HD], bqk_sb[0:HD, 1:2])
                    # K2^T into rows 64:128 (no shift)
                    nc.vector.tensor_scalar_add(
                        kt22_sb[HD:P, qs], ps[HD:P], bqk_sb[HD:P, 3:4])

                def emit_vproj(st):
                    ps = sps.tile([P, HGD], F32, tag="sc", name=f"vp{st}")
                    for kt in range(KT):
                        nc.tensor.matmul(
                            ps[:], xt_sb[:, kt, st * P:(st + 1) * P], wv_sb[:, kt, :],
                            start=(kt == 0), stop=(kt == KT - 1),
                        )
                    nc.vector.tensor_tensor(
                        v_sb[:, st, :, 0:HD], ps[:].rearrange("p (h d) -> p h d", d=HD),
                        bvr_sb[:], op=ALU.add,
                    )

                def emit_u():
                    ups = sps.tile([1, HGD], F32, tag="sc", name="ups")
                    for st in range(ST):
                        nc.tensor.matmul(
                            ups[:], ones_sb[:], v_sb[:, st, :, 0:HD],
                            start=(st == 0), stop=(st == ST - 1),
                        )
                    nc.vector.tensor_scalar_mul(
                        u_sb[0:1, :, 0:HD],
                        ups[:].rearrange("p (h d) -> p h d", d=HD), 1.0 / S)
                    nc.gpsimd.memset(u_sb[:, :, HD:HD1], 1.0)

                # ---- superunit 0 (V proj + head-2 K/Q proj interleaved) ----
                pc_lo = cps.tile([P, 1024], F32, tag="c", name="c0lo")
                pc_hi = cps.tile([P, 1024], F32, tag="c", name="c0hi")
                ers = {}
                for kt in range(ST):
                    # remaining projection chunks ride along with SU0
                    if kt == 0:
                        emit_kq_chunk(wk_sb, 2, kt01_sb, 2)
                    elif kt == 1:
                        emit_kq_chunk(wq_sb, 0, qt_sb, 2)
                    elif kt == 2:
                        emit_kq_chunk(wk_sb, 2, kt01_sb, 3)
                    elif kt == 3:
                        emit_kq_chunk(wq_sb, 0, qt_sb, 3)
                    elif kt < 8:
                        emit_kq2(kt - 4)
                    if kt == 8:
                        # duplicate K2^T into rows 0:64 for the lo stream
                        nc.gpsimd.dma_start(kt22_sb[0:HD, :], kt22_sb[HD:P, :])
                    emit_vproj(kt)
                    ers[kt] = epool.tile([P, 2, 1024], BF16, tag="e", bufs=4,
                                         name=f"e0_{kt}")
                    emit_scores_kt(0, kt, ers[kt])
                    if kt >= LAG:
                        emit_ctx_kt(0, kt - LAG, ers.pop(kt - LAG), pc_lo, pc_hi)
                for kt in range(ST - LAG, ST):
                    emit_ctx_kt(0, kt, ers.pop(kt), pc_lo, pc_hi)
                emit_u()
                emit_ctx_tails(0, pc_lo, pc_hi)
                # h0/h1 of half 0 done -> stage their AG(0) rows (hidden
                # under SU1; sync ring is idle now)
                emit_ag_in01(0, [nc.sync, nc.sync])

            # ---- superunits 1, 2 ----
            for su in (1, 2):
                pc_lo = cps.tile([P, 1024], F32, tag="c", name=f"c{su}lo")
                pc_hi = cps.tile([P, 1024], F32, tag="c", name=f"c{su}hi")
                ers = {}
                for kt in range(ST):
                    ers[kt] = epool.tile([P, 2, 1024], BF16, tag="e", bufs=4,
                                         name=f"e{su}_{kt}")
                    emit_scores_kt(su, kt, ers[kt])
                    if kt >= LAG:
                        emit_ctx_kt(su, kt - LAG, ers.pop(kt - LAG), pc_lo, pc_hi)
                for kt in range(ST - LAG, ST):
                    emit_ctx_kt(su, kt, ers.pop(kt), pc_lo, pc_hi)
                emit_ctx_tails(su, pc_lo, pc_hi)
                if su == 1:
                    # h2 of both halves done: finish half-0 payload and
                    # fire AG(0); its pull is deliberately NOT queued yet
                    # (it would gate later same-queue DMAs on AG(0)'s
                    # completion and stall the SU2 tails).
                    emit_ag_in2(0, nc.gpsimd)
                    emit_ag(0)
                    emit_ag_in2(1, nc.scalar)
            # SU2 tails done: finish half-1 payload and fire AG(1)
            emit_ag_in01(1, [nc.sync, nc.scalar])
            emit_ag(1)
            emit_pull(0, [nc.sync, nc.scalar, nc.gpsimd])

            spsx.close()   # free scores PSUM banks for the out-dense

            # ======== out dense + residual + LayerNorm (per half) ========
            ops = pools.enter_context(tc.tile_pool(name="ops", bufs=2, space="PSUM"))
            lnp = pools.enter_context(tc.tile_pool(name="lnp", bufs=1))
            h_all = lnp.tile([P, NST, H], F32)
            mu_all = lnp.tile([P, NST], F32)
            var_all = lnp.tile([P, NST], F32)
            negmu = lnp.tile([P, NST], F32)
            rstd = lnp.tile([P, NST], F32)

            def emit_dense(st4):
                ps = ops.tile([P, H], F32, tag="od", name=f"od{st4}")
                src = ctxg_sb[st4 // 2]
                c0 = (st4 % 2) * P
                for kt in range(KT):
                    lhsT = src[:, kt, c0:c0 + P]
                    nc.tensor.matmul(ps[:, 0:512], lhsT, wo_sb[:, kt, 0:512],
                                     start=(kt == 0), stop=(kt == KT - 1))
                    nc.tensor.matmul(ps[:, 512:H], lhsT, wo_sb[:, kt, 512:H],
                                     start=(kt == 0), stop=(kt == KT - 1))
                # h = out_dense + (x + bo); mean via ACT copy-accumulate
                psc = lnp.tile([P, H], F32, tag="psc", bufs=2, name=f"psc{st4}")
                nc.scalar.activation(psc[:], ps[:], AF.Identity,
                                     accum_out=mu_all[:, st4:st4 + 1])
                nc.vector.tensor_tensor(h_all[:, st4, :], psc[:],
                                        xres_sb[:, st4, :], op=ALU.add)

            def emit_ln(sts, out_eng=None):
                out_eng = out_eng or nc.sync
                s0, s1 = sts[0], sts[0] + len(sts)
                sl = slice(s0, s1)
                sq_tmp = lnp.tile([P, H], F32, tag="sq", bufs=2, name=f"sq{s0}")
                nc.vector.tensor_tensor(mu_all[:, sl], mu_all[:, sl],
                                        xsum_sb[:, sl], op=ALU.add)
                nc.vector.tensor_scalar_mul(mu_all[:, sl], mu_all[:, sl], 1.0 / H)
                nc.vector.tensor_scalar_mul(negmu[:, sl], mu_all[:, sl], -1.0)
                for st4 in sts:
                    nc.scalar.activation(sq_tmp[:], h_all[:, st4, :], AF.Square,
                                         bias=negmu[:, st4:st4 + 1],
                                         accum_out=var_all[:, st4:st4 + 1])
                nc.vector.tensor_scalar_mul(var_all[:, sl], var_all[:, sl], 1.0 / H)
                nc.vector.tensor_scalar_add(var_all[:, sl], var_all[:, sl], EPS)
                # rstd = 1/sqrt(var) with one Newton step
                std0 = lnp.tile([P, 2], F32, tag="sd", bufs=2, name=f"sd{s0}")
                nc.scalar.activation(std0[:], var_all[:, sl], AF.Sqrt)
                y0 = lnp.tile([P, 2], F32, tag="y0", bufs=2, name=f"y0{s0}")
                nc.vector.reciprocal(y0[:], std0[:])
                t0 = lnp.tile([P, 2], F32, tag="t0", bufs=2, name=f"t0{s0}")
                nc.vector.tensor_tensor(t0[:], y0[:], y0[:], op=ALU.mult)
                nc.vector.tensor_tensor(t0[:], t0[:], var_all[:, sl], op=ALU.mult)
                nc.vector.tensor_scalar_mul(t0[:], t0[:], -0.5)
                nc.vector.tensor_scalar_add(t0[:], t0[:], 1.5)
                nc.vector.tensor_tensor(rstd[:, sl], y0[:], t0[:], op=ALU.mult)
                # out = h*rstd + (-mu*rstd) in one ACT pass (gamma=1 and
                # beta=0 for this model, asserted host-side)
                nmr = lnp.tile([P, 2], F32, tag="nmr", bufs=2, name=f"nmr{s0}")
                nc.vector.tensor_tensor(nmr[:], negmu[:, sl], rstd[:, sl],
                                        op=ALU.mult)
                for st4 in sts:
                    o_sb = lnp.tile([P, H], F32, tag="o", bufs=2, name=f"o{st4}")
                    nc.scalar.activation(o_sb[:], h_all[:, st4, :], AF.Identity,
                                         bias=nmr[:, st4 - s0:st4 - s0 + 1],
                                         scale=rstd[:, st4:st4 + 1])
                    out_eng.dma_start(out_d[st4 * P:(st4 + 1) * P, :], o_sb[:])

            # half 0 (already pulled mid-SU2) overlaps A2A(1) flight
            emit_dense(0)
            emit_dense(1)
            emit_ln([0, 1], nc.scalar)
            emit_pull(1, [nc.sync, nc.scalar, nc.gpsimd])
            emit_dense(2)
            emit_dense(3)
            emit_ln([2, 3], nc.sync)

            pools.close()

    nc.compile()
    return nc


def _rows(g):
    return np.r_[QQ * g:QQ * (g + 1), 1024 + QQ * g:1024 + QQ * (g + 1)]


def _repack_kt(w):
    # [KT*P, M] -> [P, KT*M]: row p holds kt-major chunks, so the SBUF
    # tile [P, KT, M] loads as one contiguous chunk per partition
    m = w.shape[1]
    return np.ascontiguousarray(
        w.reshape(KT, P, m).transpose(1, 0, 2).reshape(P, KT * m))


def _prep_inputs(inputs):
    hs = np.asarray(inputs["hidden_states"], dtype=np.float32)
    am = np.asarray(inputs["attention_mask"], dtype=np.float32)
    Wq = np.asarray(inputs["Wq"], dtype=np.float32)
    Wk = np.asarray(inputs["Wk"], dtype=np.float32)
    Wv = np.asarray(inputs["Wv"], dtype=np.float32)
    Wo = np.asarray(inputs["Wo"], dtype=np.float32)
    bq = np.asarray(inputs["bq"], dtype=np.float32)
    bk = np.asarray(inputs["bk"], dtype=np.float32)
    bv = np.asarray(inputs["bv"], dtype=np.float32)
    bo = np.asarray(inputs["bo"], dtype=np.float32)
    lng = np.asarray(inputs["ln_gamma"], dtype=np.float32)
    lnb = np.asarray(inputs["ln_beta"], dtype=np.float32)

    wo_bf = _repack_kt(Wo.astype(ml_dtypes.bfloat16))
    # the fused LN output path folds gamma/beta away (they are constant
    # identity in this model)
    assert np.all(lng == 1.0) and np.all(lnb == 0.0)

    in_maps = []
    for c in range(NCORES):
        b, g = c // TP, c % TP
        c0 = HGD * g
        valid = am[b] >= 0
        mk = np.where(valid, 0.0, BIGNEG).astype(np.float32)
        mk2 = np.where(valid, SC_B0, SC_BM).astype(np.float32)
        gqv = np.where(valid, 0.0, BIGPOS).astype(ml_dtypes.bfloat16)[None, :]
        rows = _rows(g)
        xres = hs[b, rows] + bo
        # bias columns: bq01 | bq2 (rows 0:64) | bk01 | bk2 (rows 64:128)
        bqk = np.zeros((P, 4), dtype=np.float32)
        bqk[:, 0] = bq[c0:c0 + P]
        bqk[0:HD, 1] = bq[c0 + P:c0 + HGD]
        bqk[:, 2] = bk[c0:c0 + P]
        bqk[HD:P, 3] = bk[c0 + P:c0 + HGD]
        wkq2 = np.concatenate(
            [Wq[:, c0 + P:c0 + HGD], Wk[:, c0 + P:c0 + HGD]], axis=1)
        sm = np.empty((P, 40), dtype=np.float32)
        sm[:, 0:4] = bqk
        sm[:, 4:20] = mk.reshape(ST, P).T
        sm[:, 20:36] = mk2.reshape(ST, P).T
        sm[:, 36:40] = xres.sum(axis=1).astype(np.float32).reshape(SQ // P, P).T
        in_maps.append({
            "xt": np.ascontiguousarray(hs[b].T).astype(ml_dtypes.bfloat16),
            "xres": np.ascontiguousarray(
                xres.reshape(SQ // P, P, H).transpose(1, 0, 2)
                .reshape(P, (SQ // P) * H)),
            "wq": _repack_kt(Wq[:, c0:c0 + P].astype(ml_dtypes.bfloat16)),
            "wk": _repack_kt(Wk[:, c0:c0 + P].astype(ml_dtypes.bfloat16)),
            "wkq2": _repack_kt(wkq2.astype(ml_dtypes.bfloat16)),
            "wv": _repack_kt(Wv[:, c0:c0 + HGD].astype(ml_dtypes.bfloat16)),
            "bvr": np.ascontiguousarray(np.broadcast_to(bv[c0:c0 + HGD], (P, HGD))),
            "wo": wo_bf,
            "gq": np.ascontiguousarray(gqv),
            "sm": sm,
            "qoff": np.array([[QQ * g]], dtype=np.uint32),
        })
    return in_maps


def _run(inputs, trace=False, trace_cores=None):
    if "nc" not in _cache:
        _cache["nc"] = build()
    nc = _cache["nc"]
    in_maps = _prep_inputs(inputs)
    res = run_bass_kernel_spmd(
        nc, in_maps, list(range(NCORES)), trace=trace,
        trace_cores=trace_cores,
    )
    out = np.empty((B, S, H), dtype=np.float32)
    for c in range(NCORES):
        b, g = c // TP, c % TP
        out[b, _rows(g)] = res.results[c]["out"]
    return out, res


def kernel(**inputs) -> np.ndarray:
    out, _ = _run(inputs)
    return out
